# revision 53
# baseline (speedup 1.0000x reference)
"""Swin-style window-attention transformer block on 8 Trainium2 NeuronCores.

Data-parallel over batch B=8 (one image per core). Same 2-band-deep
software pipeline as the bf16 version (see git history / .bak), with the
four big GEMMs (qkv, proj, mlp1, mlp2) converted to fp8e4 DoubleRow
matmuls (K=256 per instruction, ~1.4x PE throughput):

  - weights quantized host-side at scale 2^9 (sigma ~10 in fp8 units);
    LN outputs scaled 2^4 (folded into the Newton-rsqrt constants), the
    attention output 2^8 (folded into the ones-matmul value), the relu
    output 2^6 (folded into the relu-evac activation scale/bias).
  - every scale/descale rides an existing op except: one bf16->fp8 cast
    per LN per band (the DMA xbar transpose is 2-byte-only, so the
    transpose runs in bf16 and the cast follows), and one extra descale
    op on the proj/mlp2 evac paths (psum holds 2^17/2^15-scaled values).
  - attention proper (QK^T, exp, AV) stays bf16: q,k are descaled at the
    qkv evac (attention scale folded there too), v stays 2^13-scaled
    with the descale folded into the softmax-normalize multiply.

Everything else (token-major LN stats via bn_stats, magic-constant
Newton rsqrt, row-packed QK^T, block-diagonal score buffer, col-packed
AV, band-interleaved MLP filler scheduling) is unchanged.
"""

import functools
import numpy as np

B, H, W, C = 8, 64, 64, 512
WH, WW = 8, 8
NH = 16
HD = C // NH
EPS = 1e-5
P = 128
NB = 8          # bands per core (window rows)
NG = 4          # 128-token groups per band (window pairs)
TB = 512        # tokens per band
N_CORES = 8

RSQRT_MAGIC = 0x5F3759DF

# fp8 scale ladder (all exact powers of two)
S_W = 2.0 ** 9     # weight quantization scale (all four GEMM weights)
S_A = 2.0 ** 4     # LN output scale (hT, h2T)
S_O = 2.0 ** 8     # attention-output scale (oT)
S_M = 2.0 ** 6     # relu-output scale (m1)
C_QKV = 2.0 ** -13          # descale for qkv psum (S_A*S_W)
C_PROJ = 2.0 ** -17         # descale for proj psum (S_O*S_W)
C_M1 = S_M * C_QKV          # relu evac scale (2^-7)
C_MLP2 = 2.0 ** -15         # descale for mlp2 psum (S_M*S_W)
ONES_VAL = 2.0 ** 13 / S_O  # makes oT come out at S_O*o (v is 2^13-scaled)


def _rel_pos_index():
    coords = np.stack(np.meshgrid(np.arange(WH), np.arange(WW), indexing="ij"))
    cf = coords.reshape(2, -1)
    rel = (cf[:, :, None] - cf[:, None, :]).transpose(1, 2, 0).copy()
    rel[..., 0] += WH - 1
    rel[..., 1] += WW - 1
    rel[..., 0] *= 2 * WW - 1
    return rel.sum(-1)  # [64, 64] int


def _emit(nc, tc, ctx, aps):
    import concourse.mybir as mybir

    dt = mybir.dt
    f32, bf16, u32, f8 = dt.float32, dt.bfloat16, dt.uint32, dt.float8e4
    AF = mybir.ActivationFunctionType
    ALU = mybir.AluOpType
    DR = mybir.MatmulPerfMode.DoubleRow

    x_r = aps["x"].rearrange(
        "(wr i) (wcp wcl j) c -> wr wcp wcl i j c", i=8, wcl=2, j=8)
    out_r = aps["out"].rearrange(
        "(wr i) (wcp wcl j) c -> wr wcp wcl i j c", i=8, wcl=2, j=8)

    wqkv_r = aps["wqkv"].rearrange("(cc p) f -> p cc f", p=P)    # [128,4,1536]
    wproj_r = aps["wproj"].rearrange("(cc p) f -> p cc f", p=P)  # [128,4,512]
    w1_r = aps["w1"].rearrange("(cc p) (fc f) -> p cc fc f", p=P, f=P)
    w2_r = aps["w2"].rearrange("(fc p) f -> p fc f", p=P)        # [128,16,512]

    const = ctx.enter_context(tc.tile_pool(name="const", bufs=1))
    xpool = ctx.enter_context(tc.tile_pool(name="xp", bufs=5))
    hpool = ctx.enter_context(tc.tile_pool(name="hp", bufs=2))
    hTpool = ctx.enter_context(tc.tile_pool(name="hTp", bufs=2))
    qkpool = ctx.enter_context(tc.tile_pool(name="qkp", bufs=2))
    vpool = ctx.enter_context(tc.tile_pool(name="vp", bufs=2))
    opool = ctx.enter_context(tc.tile_pool(name="op", bufs=2))
    m1pool = ctx.enter_context(tc.tile_pool(name="m1p", bufs=2))
    outpool = ctx.enter_context(tc.tile_pool(name="outp", bufs=2))
    spool = ctx.enter_context(tc.tile_pool(name="sp", bufs=2))
    rdpool = ctx.enter_context(tc.tile_pool(name="rdp", bufs=2))
    ptpool = ctx.enter_context(tc.tile_pool(name="ptp", bufs=2))

    # qkv/attention vs proj/MLP on separate PSUM pools so the interleaved
    # streams aren't slot-chained through each other; the attention pd/po
    # tiles ride the qkv ring (same slot size, transient per-cc lifetime)
    pp_qkv = ctx.enter_context(tc.tile_pool(name="ppqkv", bufs=2, space="PSUM"))
    pp_mlp = ctx.enter_context(tc.tile_pool(name="ppmlp", bufs=2, space="PSUM"))
    pp_s = ctx.enter_context(tc.tile_pool(name="pps", bufs=1, space="PSUM"))

    # ---- persistent constant tiles. The DMAs for the weights are NOT
    # emitted here: every DMA-transpose serializes globally against
    # in-flight DMA copies (xbar mode switch), so the weight loads must
    # come after band 0's LN1 transposes (see prologue below) ----
    wqkv_sb = const.tile([P, 4, 3 * C], f8)
    wproj_sb = const.tile([P, 4, C], f8)
    w1_sb = const.tile([P, 4, 16, P], f8)
    w2_sb = const.tile([P, 16, C], f8)
    bqkc_sb = const.tile([P, 8], f32)       # q,k bias columns (1024 feats)
    bm1c_sb = const.tile([P, 16], f32)
    bvbc_sb = const.tile([P, C], bf16)
    bpbc_sb = const.tile([P, C], bf16)
    bm2bc_sb = const.tile([P, C], bf16)
    biasT_sb = const.tile([P, NH, 64], bf16)
    ones_sb = const.tile([P, 32], bf16)
    nc.vector.memset(ones_sb[:], ONES_VAL)
    # persistent block-diagonal score buffer [key(128), head, pair, wcl, 64]
    scores_sb = const.tile([P, NH, NG, 2, 64], bf16)
    nc.gpsimd.memset(scores_sb[:], 0.0)

    def load_weights():
        # on the sync ring: the FIFO behind band 0's transposes defers
        # them naturally, so they neither race band-0 x loads for HBM nor
        # cross-queue-stall the xbar transposes
        nc.sync.dma_start(wqkv_sb[:, :, 2 * C:], wqkv_r[:, :, 2 * C:])
        nc.sync.dma_start(bqkc_sb[:], aps["bqkc"])
        nc.sync.dma_start(bvbc_sb[:], aps["bvbc"])
        nc.sync.dma_start(biasT_sb[:], aps["biasT"])
        nc.sync.dma_start(wproj_sb[:], wproj_r)
        nc.sync.dma_start(bpbc_sb[:], aps["bpbc"])

    def load_weights_mlp():
        # MLP-era weights, first needed at attention(1)'s mlp1(0) fillers:
        # emitted behind band 0's T1(1) transposes in the sync FIFO
        nc.sync.dma_start(w1_sb[:], w1_r)
        nc.sync.dma_start(w2_sb[:], w2_r)
        nc.sync.dma_start(bm1c_sb[:], aps["bm1c"])
        nc.sync.dma_start(bm2bc_sb[:], aps["bm2bc"])

    def load_stats(band, prologue=False):
        # x loads ride HWDGE rings, NOT the gpsimd SWDGE ring: the xbar
        # transposes serialize against in-flight SWDGE copies, and with
        # fp8 there is no PE work left to hide that wait behind. The
        # prologue splits band-0 loads per-wcl across sync+scalar to
        # shorten the serial startup chain.
        x_sb = xpool.tile([P, NG, C], f32, tag="x")
        st = spool.tile([P, NG, 6], f32, tag="st1", name="st1")
        mv = spool.tile([P, NG, 2], f32, tag="mv1", name="mv1")
        for g in range(NG):
            for wcl in range(2):
                eng = nc.sync if wcl == 0 else nc.scalar
                eng.dma_start(x_sb[wcl * 64:(wcl + 1) * 64, g],
                              x_r[band, g, wcl])
            if prologue:
                nc.vector.bn_stats(out=st[:, g], in_=x_sb[:, g])
                nc.vector.bn_aggr(out=mv[:, g], in_=st[:, g])
            else:
                # these stats are ~2 bands ahead of their consumer:
                # float them to the back of the DVE queue so they don't
                # delay the current band's bias/exp/evac chain
                with tc.high_priority(offset=-350):
                    nc.vector.bn_stats(out=st[:, g], in_=x_sb[:, g])
                    nc.vector.bn_aggr(out=mv[:, g], in_=st[:, g])
        return x_sb, mv

    def ln_rstd(mv, nm):
        # a = S_A*rsqrt(var+eps), b = -mean*a on DVE (magic-constant
        # Newton, no ACT table-set switch); the fp8 activation scale S_A
        # is folded into the final Newton constants
        a_t = spool.tile([P, NG], f32, tag=f"a{nm}", name=f"a{nm}")
        b_t = spool.tile([P, NG], f32, tag=f"b{nm}", name=f"b{nm}")
        t_t = spool.tile([P, NG], f32, tag=f"t{nm}", name=f"t{nm}")
        nc.vector.tensor_scalar_add(b_t[:], mv[:, :, 1], EPS)   # ve = var+eps
        yu, vu = a_t[:].bitcast(u32), b_t[:].bitcast(u32)
        nc.vector.tensor_scalar(yu, vu, 1, None, ALU.logical_shift_right)
        nc.vector.tensor_scalar(yu, yu, RSQRT_MAGIC ^ 0xFFFFFFFF, None, ALU.add)
        nc.vector.tensor_scalar(yu, yu, 0xFFFFFFFF, None, ALU.bitwise_xor)
        # Newton: a *= S_A*(1.5 - 0.5*ve*a^2) (~0.1% rstd err)
        nc.vector.tensor_tensor(t_t[:], a_t[:], a_t[:], ALU.mult)
        nc.vector.tensor_tensor(t_t[:], t_t[:], b_t[:], ALU.mult)
        nc.vector.tensor_scalar(t_t[:], t_t[:], -0.5 * S_A, 1.5 * S_A,
                                ALU.mult, ALU.add)
        nc.vector.tensor_tensor(a_t[:], a_t[:], t_t[:], ALU.mult)
        nc.vector.tensor_tensor(b_t[:], mv[:, :, 0], a_t[:], ALU.mult)
        nc.vector.tensor_scalar_mul(b_t[:], b_t[:], -1.0)
        return a_t, b_t

    def ln_apply_transpose(x_sb, ab, htag):
        # apply alternating ACT/DVE (parallel engines halve the stage),
        # then batched xbar transpose per group on the sync ring (bf16:
        # the xbar can't move 1-byte elements), then a cast to fp8
        a_t, b_t = ab
        h_sb = hpool.tile([P, NG, C], bf16, tag="h")
        hT_sb = hTpool.tile([P, 4, TB], bf16, tag=htag + "b")
        h8_sb = hTpool.tile([P, 4, TB], f8, tag=htag)
        for g in range(NG):
            if g % 2 == 0:
                nc.scalar.activation(h_sb[:, g], x_sb[:, g], AF.Identity,
                                     bias=b_t[:, g:g + 1],
                                     scale=a_t[:, g:g + 1])
            else:
                nc.vector.tensor_scalar(
                    h_sb[:, g], x_sb[:, g], a_t[:, g:g + 1], b_t[:, g:g + 1],
                    ALU.mult, ALU.add)
            nc.sync.dma_start(hT_sb[:, :, g * P:(g + 1) * P], h_sb[:, g],
                              transpose=True)
            # per-group cast so the last cast chunk, not a whole-tile op,
            # trails the last transpose; hT on DVE, h2T on ACT
            if htag == "hT":
                nc.vector.tensor_scalar_mul(
                    h8_sb[:, :, g * P:(g + 1) * P],
                    hT_sb[:, :, g * P:(g + 1) * P], 1.0)
            else:
                nc.scalar.copy(h8_sb[:, :, g * P:(g + 1) * P],
                               hT_sb[:, :, g * P:(g + 1) * P])
        return h8_sb

    def qk_chunk(qk_sb, hT_sb, fs):
        cq, ck = C_QKV * HD ** -0.5, C_QKV
        for f in fs:
            ps = pp_qkv.tile([P, TB], f32, tag="big")
            for t in range(2):
                nc.tensor.matmul(
                    ps[:], wqkv_sb[:, 2 * t:2 * t + 2, f * P:(f + 1) * P],
                    hT_sb[:, 2 * t:2 * t + 2, :],
                    start=(t == 0), stop=(t == 1), perf_mode=DR)
            qi, ci = divmod(f, 4)
            nc.any.tensor_scalar(qk_sb[:, qi, ci], ps[:],
                                 cq if qi == 0 else ck,
                                 bqkc_sb[:, f:f + 1], ALU.mult, ALU.add)

    def v_chunk(v_sb, hT_sb, gs):
        for g in gs:
            ps = pp_qkv.tile([P, C], f32, tag="big")
            for t in range(2):
                nc.tensor.matmul(
                    ps[:], hT_sb[:, 2 * t:2 * t + 2, g * P:(g + 1) * P],
                    wqkv_sb[:, 2 * t:2 * t + 2, 2 * C:3 * C],
                    start=(t == 0), stop=(t == 1), perf_mode=DR)
            # v stays 2^13-scaled (descale folded into ONES_VAL)
            nc.any.tensor_tensor(v_sb[:, g], ps[:], bvbc_sb[:], ALU.add)

    def emit_qkv(hT_sb):
        qk_sb = qkpool.tile([P, 2, 4, TB], bf16, tag="qk")
        qk_chunk(qk_sb, hT_sb, range(8))
        v_sb = vpool.tile([P, NG, C], bf16, tag="v")
        v_chunk(v_sb, hT_sb, range(NG))
        return qk_sb, v_sb

    def attn_cc(cc, qk_sb, v_sb, oT_sb, pe_filler):
        # QK^T for 4 heads, one PSUM bank per head-tile i (concurrent
        # row-packed matmuls must drain into distinct banks); only the
        # first 256 fp32 of each bank are used.
        pss = pp_s.tile([P, 4, TB], f32, tag="pss")
        psv = pss[:, :, 0:NG * 64].rearrange("p i (g q) -> p i g q", g=NG)
        for w in range(8):
            pairi, wcl = divmod(w, 2)
            for i in range(4):
                out_sl = (psv[0:64, i, pairi] if wcl == 0
                          else psv[64:128, i, pairi])
                nc.tensor.matmul(
                    out_sl,
                    qk_sb[32 * i:32 * (i + 1), 1, cc, w * 64:(w + 1) * 64],
                    qk_sb[32 * i:32 * (i + 1), 0, cc, w * 64:(w + 1) * 64],
                    start=True, stop=True,
                    tile_position=(32 * i, 0 if wcl == 0 else 64))
        # bias+exp priority-boosted: on the ACT/DVE queues they must jump
        # ahead of the filler relu evacs and LN-chain casts emitted around
        # them, since exp(cc) gates the next cc's QK via the psv slot
        with tc.high_priority(offset=250):
            nc.vector.tensor_tensor(
                psv[:], psv[:],
                biasT_sb[:, 4 * cc:4 * cc + 4, None, :].to_broadcast(
                    (P, 4, NG, 64)),
                ALU.add)
            nc.scalar.activation(scores_sb[0:64, 4 * cc:4 * cc + 4, :, 0, :],
                                 psv[0:64], AF.Exp)
            nc.scalar.activation(scores_sb[64:128, 4 * cc:4 * cc + 4, :, 1, :],
                                 psv[64:128], AF.Exp)
        # PE filler work (mlp of band-2) lands here in the PE stream, so
        # the engine computes through the bias+exp chain instead of
        # stalling on the pd/AV matmuls that need the fresh scores
        pe_filler()
        # softmax denominators: col-packed ones-matmuls + fast reciprocal
        pd = pp_qkv.tile([P, NG, P], f32, tag="big")
        for pair in range(NG):
            for j in range(4):
                nc.tensor.matmul(
                    pd[32 * j:32 * (j + 1), pair], ones_sb[:],
                    scores_sb[:, 4 * cc + j, pair],
                    start=True, stop=True, tile_position=(0, 32 * j))
        rd = rdpool.tile([P, NG, P], f32, tag="rd")
        with tc.high_priority(offset=250):
            nc.vector.reciprocal_approx_fast(rd[:], pd[:])
        # AV col-packed 4 heads into one PSUM bank; normalize (and the
        # 2^13->2^8 rescale, via ONES_VAL) on the PSUM->SBUF evacuation
        # multiply, writing fp8 directly
        po = pp_qkv.tile([P, NG, P], f32, tag="big")
        for pair in range(NG):
            for j in range(4):
                nc.tensor.matmul(
                    po[32 * j:32 * (j + 1), pair],
                    v_sb[:, pair, (4 * cc + j) * HD:(4 * cc + j + 1) * HD],
                    scores_sb[:, 4 * cc + j, pair],
                    start=True, stop=True, tile_position=(0, 32 * j))
        with tc.high_priority(offset=250):
            nc.vector.tensor_tensor(oT_sb[:, cc], po[:], rd[:], ALU.mult)

    def mlp1_chunk(m1_sb, h2T_sb, fcs):
        for fc in fcs:
            ps = pp_mlp.tile([P, TB], f32, tag="mlp")
            for t in range(2):
                nc.tensor.matmul(
                    ps[:], w1_sb[:, 2 * t:2 * t + 2, fc, :],
                    h2T_sb[:, 2 * t:2 * t + 2, :],
                    start=(t == 0), stop=(t == 1), perf_mode=DR)
            # psum is 2^13-scaled; bias column is host-scaled by S_M so
            # the evac writes S_M*relu(true) as fp8 in one op
            nc.scalar.activation(m1_sb[:, fc], ps[:], AF.Relu,
                                 bias=bm1c_sb[:, fc:fc + 1], scale=C_M1)

    def mlp2_g(prev, g):
        m1_sb, y_sb, pband = prev
        ps = pp_mlp.tile([P, C], f32, tag="mlp")
        for t in range(8):
            nc.tensor.matmul(
                ps[:], m1_sb[:, 2 * t:2 * t + 2, g * P:(g + 1) * P],
                w2_sb[:, 2 * t:2 * t + 2, :],
                start=(t == 0), stop=(t == 7), perf_mode=DR)
        t_sb = ptpool.tile([P, C], f32, tag="mt")
        o_sb = outpool.tile([P, C], f32, tag="out")
        nc.any.tensor_scalar_mul(t_sb[:], ps[:], C_MLP2)
        nc.any.tensor_tensor(o_sb[:], t_sb[:], y_sb[:, g], ALU.add)
        nc.any.tensor_tensor(o_sb[:], o_sb[:], bm2bc_sb[:], ALU.add)
        # out stores on the sync HWDGE ring (not SWDGE: xbar conflict;
        # hwdge copies were observed NOT to block transposes)
        for wcl in range(2):
            nc.sync.dma_start(out_r[pband, g, wcl],
                              o_sb[wcl * 64:(wcl + 1) * 64])

    # ---- prologue: band 0 LN1 runs BEFORE the bulk weight loads so its
    # x loads and transposes aren't starved of HBM bandwidth / xbar turns;
    # only the q,k weights (which gate the first GEMM) go first. The LN1
    # chain runs TWO bands ahead of its qkv consumer (the apply/transpose/
    # cast chain is ~15us of serial latency incl. xbar-vs-DMA waits; one
    # band of lookahead wasn't enough to hide it once fp8 shrank the PE
    # work per band), so the prologue also builds hT8(1) ----
    nc.sync.dma_start(wqkv_sb[:, :, :2 * C], wqkv_r[:, :, :2 * C])
    xs = {}
    x_sb, mv1 = load_stats(0, prologue=True)
    xs[0] = x_sb
    # band-0 chain FIRST: its transposes serialize against every
    # in-flight DMA, so x(1) and the weight bulk are emitted after it
    hT8 = {0: ln_apply_transpose(x_sb, ln_rstd(mv1, "1"), "hT")}
    load_weights()
    qk_sb, v_sb = emit_qkv(hT8[0])
    xs[1], mv1b = load_stats(1)
    hT8[1] = ln_apply_transpose(xs[1], ln_rstd(mv1b, "1"), "hT")
    pending = {}  # band -> (h2T8, y, band): MLP not yet run

    for band in range(NB):
        # band+2's x loads + LN stats go first: the rings and DVE
        # have slack at the start of the attention block
        if band + 2 < NB:
            xs[band + 2], nxt_mv = load_stats(band + 2)

        # ---- attention(band), interleaving mlp1(band-2) + mlp2(band-2)
        # as PE fillers. The MLP trails its band by TWO bands so the
        # LN2 chain (proj evac -> stats -> rstd -> apply -> xbar
        # transpose -> cast, ~15-20us of serial latency incl. the
        # transpose-vs-in-flight-DMA waits) has a full band of slack
        # before the mlp1 fillers consume h2T8. The four chunks are
        # mlp1 halves then mlp2 halves (mlp2 needs all of m1, which
        # fillers 0-1 complete) ----
        x_sb = xs[band]
        oT_sb = opool.tile([P, 4, TB], f8, tag="oT")
        if band - 2 in pending:
            h2T_prev, y_prev, pband = pending.pop(band - 2)
            m1_sb = m1pool.tile([P, 16, TB], f8, tag="m1")
            pm = (m1_sb, y_prev, pband)
            fillers = [lambda: mlp1_chunk(m1_sb, h2T_prev, range(0, 8)),
                       lambda: mlp1_chunk(m1_sb, h2T_prev, range(8, 16)),
                       lambda: (mlp2_g(pm, 0), mlp2_g(pm, 1)),
                       lambda: (mlp2_g(pm, 2), mlp2_g(pm, 3))]
        else:
            fillers = [lambda: None] * 4
        attn_cc(0, qk_sb, v_sb, oT_sb, fillers[0])
        attn_cc(1, qk_sb, v_sb, oT_sb, fillers[1])
        if band + 2 < NB:
            nxt_ab = ln_rstd(nxt_mv, "1")
        attn_cc(2, qk_sb, v_sb, oT_sb, fillers[2])
        attn_cc(3, qk_sb, v_sb, oT_sb, fillers[3])
        # LN1(band+2) applies + transposes: ACT/DVE/sync are free once the
        # last exps are issued; hT8(band+2) has a whole band of slack
        # before qkv(band+2) consumes it
        if band + 2 < NB:
            hT8[band + 2] = ln_apply_transpose(xs[band + 2], nxt_ab, "hT")

        # ---- proj(band) + residual + LN2 stats ----
        st2 = spool.tile([P, NG, 6], f32, tag="st2", name="st2")
        mv2 = spool.tile([P, NG, 2], f32, tag="mv2", name="mv2")
        for g in range(NG):
            ps = pp_mlp.tile([P, C], f32, tag="mlp")
            for t in range(2):
                nc.tensor.matmul(
                    ps[:], oT_sb[:, 2 * t:2 * t + 2, g * P:(g + 1) * P],
                    wproj_sb[:, 2 * t:2 * t + 2, :],
                    start=(t == 0), stop=(t == 1), perf_mode=DR)
            t_sb = ptpool.tile([P, C], f32, tag="pt")
            nc.any.tensor_scalar_mul(t_sb[:], ps[:], C_PROJ)
            # everything past the psum-freeing descale only feeds the
            # LN2 chain, which has a spare band: float it behind the
            # current band's bias/exp/evac work on DVE/ACT
            with tc.high_priority(offset=-150):
                nc.any.tensor_tensor(x_sb[:, g], t_sb[:], x_sb[:, g],
                                     ALU.add)
                nc.any.tensor_tensor(x_sb[:, g], x_sb[:, g], bpbc_sb[:],
                                     ALU.add)
                nc.vector.bn_stats(out=st2[:, g], in_=x_sb[:, g])
                nc.vector.bn_aggr(out=mv2[:, g], in_=st2[:, g])
        y_sb = x_sb

        # ---- qkv(band+1) keeps the PE stream unbroken while the LN2
        # chain below runs on DVE/ACT/sync ----
        if band + 1 < NB:
            qk_sb, v_sb = emit_qkv(hT8[band + 1])
        h2T_sb = ln_apply_transpose(y_sb, ln_rstd(mv2, "2"), "h2T")
        if band == 0:
            # MLP-era weights behind T2(0) in the sync FIFO: ready well
            # before attention(2)'s mlp1(0) fillers, without delaying the
            # T1/T2 transposes behind weight traffic
            load_weights_mlp()

        pending[band] = (h2T_sb, y_sb, band)

    # ---- epilogue: the last two bands' MLPs (their attention cover is
    # gone); mlp(NB-2) starts immediately and hides the LN2(NB-1) chain ----
    for pband in (NB - 2, NB - 1):
        h2T_p, y_p, _ = pending.pop(pband)
        m1_sb = m1pool.tile([P, 16, TB], f8, tag="m1")
        mlp1_chunk(m1_sb, h2T_p, range(16))
        for g in range(NG):
            mlp2_g((m1_sb, y_p, pband), g)


@functools.lru_cache(maxsize=2)
def _build():
    from contextlib import ExitStack
    import concourse.mybir as mybir
    import concourse.tile as tile
    from concourse import bacc

    dt = mybir.dt
    nc = bacc.Bacc("TRN2", target_bir_lowering=False, debug=False,
                   num_devices=N_CORES)
    aps = {}
    specs = [
        ("x", [H, W, C], dt.float32),
        ("wqkv", [C, 3 * C], dt.float8e4),
        ("wproj", [C, C], dt.float8e4),
        ("w1", [C, 4 * C], dt.float8e4),
        ("w2", [4 * C, C], dt.float8e4),
        ("bqkc", [P, 8], dt.float32),
        ("bm1c", [P, 16], dt.float32),
        ("bvbc", [P, C], dt.bfloat16),
        ("bpbc", [P, C], dt.bfloat16),
        ("bm2bc", [P, C], dt.bfloat16),
        ("biasT", [P, NH, 64], dt.bfloat16),
    ]
    for name, shape, dtype in specs:
        aps[name] = nc.dram_tensor(name, shape, dtype,
                                   kind="ExternalInput").ap()
    aps["out"] = nc.dram_tensor("out", [H, W, C], dt.float32,
                                kind="ExternalOutput").ap()
    with tile.TileContext(nc) as tc:
        with ExitStack() as ctx:
            _emit(nc, tc, ctx, aps)
    nc.compile()
    return nc


def _prepare_in_maps(x, g1, b1, wqkv, bqkv, wproj, bproj, rel_bias, g2, b2,
                     w1, bm1, w2, bm2):
    x = np.asarray(x, np.float32)
    f = lambda a: np.ascontiguousarray(np.asarray(a, np.float32))
    g1, b1, wqkv, bqkv = f(g1), f(b1), f(wqkv), f(bqkv)
    wproj, bproj, rel_bias = f(wproj), f(bproj), f(rel_bias)
    g2, b2, w1, bm1, w2, bm2 = f(g2), f(b2), f(w1), f(bm1), f(w2), f(bm2)

    # fold LN1 affine into wqkv/bqkv. The attention scale HD^-0.5 is NOT
    # folded into the q weights (that would shift their fp8 binades for
    # nothing) -- it rides the q evac descale constant instead; the bias
    # columns DO carry it since they're added post-descale.
    wqkv_f = g1[:, None] * wqkv
    bqkv_f = b1 @ wqkv + bqkv
    sc = HD ** -0.5
    bqkv_f[:C] *= sc
    # fold LN2 affine into w1/bm1
    w1_f = g2[:, None] * w1
    bm1_f = b2 @ w1 + bm1

    bqkc = np.ascontiguousarray(bqkv_f[:2 * C].reshape(8, P).T)   # [128, 8]
    # bm1 is added inside the relu evac, post-descale but pre-S_M-rescale
    bm1c = np.ascontiguousarray((bm1_f * S_M).reshape(16, P).T)   # [128, 16]
    import ml_dtypes
    bfarr = lambda a: np.ascontiguousarray(a).astype(ml_dtypes.bfloat16)
    fp8arr = lambda a: np.clip(np.ascontiguousarray(a) * S_W, -240.0,
                               240.0).astype(ml_dtypes.float8_e4m3)
    # v bias is 2^13-scaled: v lives in 2^13 units until the softmax-
    # normalize multiply (ONES_VAL folds the descale)
    bvbc = bfarr(np.broadcast_to(bqkv_f[2 * C:] * (S_A * S_W), (P, C)))
    bpbc = bfarr(np.broadcast_to(bproj, (P, C)))
    bm2bc = bfarr(np.broadcast_to(bm2, (P, C)))

    idx = _rel_pos_index()                              # [64(n), 64(m)]
    bias_nm = rel_bias[idx, :]                          # [n, m, NH]
    biasT_h = bias_nm.transpose(2, 1, 0)                # [NH, m, n]
    biasT = np.concatenate([biasT_h, biasT_h], axis=1)  # [NH, 128, 64]
    biasT = bfarr(biasT.transpose(1, 0, 2))             # [128, NH, 64]

    wqkv_b, wproj_b, w1_b, w2_b = (fp8arr(wqkv_f), fp8arr(wproj),
                                   fp8arr(w1_f), fp8arr(w2))
    shared = dict(wqkv=wqkv_b, wproj=wproj_b, w1=w1_b, w2=w2_b,
                  bqkc=bqkc, bm1c=bm1c, bvbc=bvbc, bpbc=bpbc, bm2bc=bm2bc,
                  biasT=biasT)
    return [dict(x=np.ascontiguousarray(x[c]), **shared)
            for c in range(N_CORES)]


def kernel(**inputs):
    from concourse.bass_utils import run_bass_kernel_spmd

    in_maps = _prepare_in_maps(**inputs)
    nc = _build()
    res = run_bass_kernel_spmd(nc, in_maps, core_ids=list(range(N_CORES)))
    return np.stack([res.results[c]["out"] for c in range(N_CORES)], axis=0)


# revision 55
# speedup vs baseline: 1.2167x; 1.2167x over previous
"""Swin-style window-attention transformer block on 8 Trainium2 NeuronCores.

Data-parallel over batch B=8 (one image per core). Same 2-band-deep
software pipeline as the bf16 version (see git history / .bak), with the
four big GEMMs (qkv, proj, mlp1, mlp2) converted to fp8e4 DoubleRow
matmuls (K=256 per instruction, ~1.4x PE throughput):

  - weights quantized host-side at scale 2^9 (sigma ~10 in fp8 units);
    LN outputs scaled 2^4 (folded into the Newton-rsqrt constants), the
    attention output 2^8 (folded into the ones-matmul value), the relu
    output 2^6 (folded into the relu-evac activation scale/bias).
  - every scale/descale rides an existing op except: one bf16->fp8 cast
    per LN per band (the DMA xbar transpose is 2-byte-only, so the
    transpose runs in bf16 and the cast follows), and one extra descale
    op on the proj/mlp2 evac paths (psum holds 2^17/2^15-scaled values).
  - attention proper (QK^T, exp, AV) stays bf16: q,k are descaled at the
    qkv evac (attention scale folded there too), v stays 2^13-scaled
    with the descale folded into the softmax-normalize multiply.

Everything else (token-major LN stats via bn_stats, magic-constant
Newton rsqrt, row-packed QK^T, block-diagonal score buffer, col-packed
AV, band-interleaved MLP filler scheduling) is unchanged.
"""

import functools
import numpy as np

B, H, W, C = 8, 64, 64, 512
WH, WW = 8, 8
NH = 16
HD = C // NH
EPS = 1e-5
P = 128
NB = 8          # bands per core (window rows)
NG = 4          # 128-token groups per band (window pairs)
TB = 512        # tokens per band
N_CORES = 8

RSQRT_MAGIC = 0x5F3759DF

# fp8 scale ladder (all exact powers of two)
S_W = 2.0 ** 9     # weight quantization scale (all four GEMM weights)
S_A = 2.0 ** 4     # LN output scale (hT, h2T)
S_O = 2.0 ** 8     # attention-output scale (oT)
S_M = 2.0 ** 6     # relu-output scale (m1)
C_QKV = 2.0 ** -13          # descale for qkv psum (S_A*S_W)
C_PROJ = 2.0 ** -17         # descale for proj psum (S_O*S_W)
C_M1 = S_M * C_QKV          # relu evac scale (2^-7)
C_MLP2 = 2.0 ** -15         # descale for mlp2 psum (S_M*S_W)
ONES_VAL = 2.0 ** 13 / S_O  # makes oT come out at S_O*o (v is 2^13-scaled)


def _rel_pos_index():
    coords = np.stack(np.meshgrid(np.arange(WH), np.arange(WW), indexing="ij"))
    cf = coords.reshape(2, -1)
    rel = (cf[:, :, None] - cf[:, None, :]).transpose(1, 2, 0).copy()
    rel[..., 0] += WH - 1
    rel[..., 1] += WW - 1
    rel[..., 0] *= 2 * WW - 1
    return rel.sum(-1)  # [64, 64] int


def _emit(nc, tc, ctx, aps):
    import concourse.mybir as mybir

    dt = mybir.dt
    f32, bf16, u32, f8 = dt.float32, dt.bfloat16, dt.uint32, dt.float8e4
    AF = mybir.ActivationFunctionType
    ALU = mybir.AluOpType
    DR = mybir.MatmulPerfMode.DoubleRow

    x_r = aps["x"].rearrange(
        "(wr i) (wcp wcl j) c -> wr wcp wcl i j c", i=8, wcl=2, j=8)
    out_r = aps["out"].rearrange(
        "(wr i) (wcp wcl j) c -> wr wcp wcl i j c", i=8, wcl=2, j=8)

    wqkv_r = aps["wqkv"].rearrange("(cc p) f -> p cc f", p=P)    # [128,4,1536]
    wproj_r = aps["wproj"].rearrange("(cc p) f -> p cc f", p=P)  # [128,4,512]
    w1_r = aps["w1"].rearrange("(cc p) (fc f) -> p cc fc f", p=P, f=P)
    w2_r = aps["w2"].rearrange("(fc p) f -> p fc f", p=P)        # [128,16,512]

    const = ctx.enter_context(tc.tile_pool(name="const", bufs=1))
    xpool = ctx.enter_context(tc.tile_pool(name="xp", bufs=5))
    hpool = ctx.enter_context(tc.tile_pool(name="hp", bufs=2))
    hTpool = ctx.enter_context(tc.tile_pool(name="hTp", bufs=2))
    qkpool = ctx.enter_context(tc.tile_pool(name="qkp", bufs=2))
    vpool = ctx.enter_context(tc.tile_pool(name="vp", bufs=2))
    opool = ctx.enter_context(tc.tile_pool(name="op", bufs=2))
    m1pool = ctx.enter_context(tc.tile_pool(name="m1p", bufs=2))
    outpool = ctx.enter_context(tc.tile_pool(name="outp", bufs=2))
    spool = ctx.enter_context(tc.tile_pool(name="sp", bufs=2))
    rdpool = ctx.enter_context(tc.tile_pool(name="rdp", bufs=2))
    ptpool = ctx.enter_context(tc.tile_pool(name="ptp", bufs=2))

    # qkv/attention vs proj/MLP on separate PSUM pools so the interleaved
    # streams aren't slot-chained through each other; the attention pd/po
    # tiles ride the qkv ring (same slot size, transient per-cc lifetime)
    pp_qkv = ctx.enter_context(tc.tile_pool(name="ppqkv", bufs=2, space="PSUM"))
    pp_mlp = ctx.enter_context(tc.tile_pool(name="ppmlp", bufs=2, space="PSUM"))
    pp_s = ctx.enter_context(tc.tile_pool(name="pps", bufs=1, space="PSUM"))

    # ---- persistent constant tiles. The DMAs for the weights are NOT
    # emitted here: every DMA-transpose serializes globally against
    # in-flight DMA copies (xbar mode switch), so the weight loads must
    # come after band 0's LN1 transposes (see prologue below) ----
    wqkv_sb = const.tile([P, 4, 3 * C], f8)
    wproj_sb = const.tile([P, 4, C], f8)
    w1_sb = const.tile([P, 4, 16, P], f8)
    w2_sb = const.tile([P, 16, C], f8)
    bqkc_sb = const.tile([P, 8], f32)       # q,k bias columns (1024 feats)
    bm1c_sb = const.tile([P, 16], f32)
    bvbc_sb = const.tile([P, C], bf16)
    bpbc_sb = const.tile([P, C], bf16)
    bm2bc_sb = const.tile([P, C], bf16)
    biasT_sb = const.tile([P, NH, 64], bf16)
    ones_sb = const.tile([P, 32], bf16)
    nc.vector.memset(ones_sb[:], ONES_VAL)
    # persistent block-diagonal score buffer [key(128), head, pair, wcl, 64]
    scores_sb = const.tile([P, NH, NG, 2, 64], bf16)
    nc.gpsimd.memset(scores_sb[:], 0.0)

    def load_weights():
        # on the sync ring: the FIFO behind band 0's transposes defers
        # them naturally, so they neither race band-0 x loads for HBM nor
        # cross-queue-stall the xbar transposes
        nc.sync.dma_start(wqkv_sb[:, :, 2 * C:], wqkv_r[:, :, 2 * C:])
        nc.sync.dma_start(bqkc_sb[:], aps["bqkc"])
        nc.sync.dma_start(bvbc_sb[:], aps["bvbc"])
        nc.sync.dma_start(biasT_sb[:], aps["biasT"])
        nc.sync.dma_start(wproj_sb[:], wproj_r)
        nc.sync.dma_start(bpbc_sb[:], aps["bpbc"])

    def load_weights_mlp():
        # MLP-era weights, first needed at attention(1)'s mlp1(0) fillers:
        # emitted behind band 0's T1(1) transposes in the sync FIFO
        nc.sync.dma_start(w1_sb[:], w1_r)
        nc.sync.dma_start(w2_sb[:], w2_r)
        nc.sync.dma_start(bm1c_sb[:], aps["bm1c"])
        nc.sync.dma_start(bm2bc_sb[:], aps["bm2bc"])

    def load_stats(band, prologue=False):
        # x loads ride HWDGE rings, NOT the gpsimd SWDGE ring: the xbar
        # transposes serialize against in-flight SWDGE copies, and with
        # fp8 there is no PE work left to hide that wait behind. The
        # prologue splits band-0 loads per-wcl across sync+scalar to
        # shorten the serial startup chain.
        x_sb = xpool.tile([P, NG, C], f32, tag="x")
        st = spool.tile([P, NG, 6], f32, tag="st1", name="st1")
        mv = spool.tile([P, NG, 2], f32, tag="mv1", name="mv1")
        for g in range(NG):
            for wcl in range(2):
                eng = nc.sync if wcl == 0 else nc.scalar
                eng.dma_start(x_sb[wcl * 64:(wcl + 1) * 64, g],
                              x_r[band, g, wcl])
            if prologue:
                nc.vector.bn_stats(out=st[:, g], in_=x_sb[:, g])
                nc.vector.bn_aggr(out=mv[:, g], in_=st[:, g])
            else:
                # these stats are ~2 bands ahead of their consumer:
                # float them to the back of the DVE queue so they don't
                # delay the current band's bias/exp/evac chain
                with tc.high_priority(offset=-350):
                    nc.vector.bn_stats(out=st[:, g], in_=x_sb[:, g])
                    nc.vector.bn_aggr(out=mv[:, g], in_=st[:, g])
        return x_sb, mv

    def ln_rstd(mv, nm):
        # a = S_A*rsqrt(var+eps), b = -mean*a on DVE (magic-constant
        # Newton, no ACT table-set switch); the fp8 activation scale S_A
        # is folded into the final Newton constants
        a_t = spool.tile([P, NG], f32, tag=f"a{nm}", name=f"a{nm}")
        b_t = spool.tile([P, NG], f32, tag=f"b{nm}", name=f"b{nm}")
        t_t = spool.tile([P, NG], f32, tag=f"t{nm}", name=f"t{nm}")
        nc.vector.tensor_scalar_add(b_t[:], mv[:, :, 1], EPS)   # ve = var+eps
        yu, vu = a_t[:].bitcast(u32), b_t[:].bitcast(u32)
        nc.vector.tensor_scalar(yu, vu, 1, None, ALU.logical_shift_right)
        nc.vector.tensor_scalar(yu, yu, RSQRT_MAGIC ^ 0xFFFFFFFF, None, ALU.add)
        nc.vector.tensor_scalar(yu, yu, 0xFFFFFFFF, None, ALU.bitwise_xor)
        # Newton: a *= S_A*(1.5 - 0.5*ve*a^2) (~0.1% rstd err)
        nc.vector.tensor_tensor(t_t[:], a_t[:], a_t[:], ALU.mult)
        nc.vector.tensor_tensor(t_t[:], t_t[:], b_t[:], ALU.mult)
        nc.vector.tensor_scalar(t_t[:], t_t[:], -0.5 * S_A, 1.5 * S_A,
                                ALU.mult, ALU.add)
        nc.vector.tensor_tensor(a_t[:], a_t[:], t_t[:], ALU.mult)
        nc.vector.tensor_tensor(b_t[:], mv[:, :, 0], a_t[:], ALU.mult)
        nc.vector.tensor_scalar_mul(b_t[:], b_t[:], -1.0)
        return a_t, b_t

    def ln_apply_transpose(x_sb, ab, htag):
        # apply alternating ACT/DVE (parallel engines halve the stage),
        # then batched xbar transpose per group on the sync ring (bf16:
        # the xbar can't move 1-byte elements), then a cast to fp8
        a_t, b_t = ab
        h_sb = hpool.tile([P, NG, C], bf16, tag="h")
        hT_sb = hTpool.tile([P, 4, TB], bf16, tag=htag + "b")
        h8_sb = hTpool.tile([P, 4, TB], f8, tag=htag)
        for g in range(NG):
            if g % 2 == 0:
                nc.scalar.activation(h_sb[:, g], x_sb[:, g], AF.Identity,
                                     bias=b_t[:, g:g + 1],
                                     scale=a_t[:, g:g + 1])
            else:
                nc.vector.tensor_scalar(
                    h_sb[:, g], x_sb[:, g], a_t[:, g:g + 1], b_t[:, g:g + 1],
                    ALU.mult, ALU.add)
            nc.sync.dma_start(hT_sb[:, :, g * P:(g + 1) * P], h_sb[:, g],
                              transpose=True)
            # per-group cast so the last cast chunk, not a whole-tile op,
            # trails the last transpose; hT on DVE, h2T on ACT
            if htag == "hT":
                nc.vector.tensor_scalar_mul(
                    h8_sb[:, :, g * P:(g + 1) * P],
                    hT_sb[:, :, g * P:(g + 1) * P], 1.0)
            else:
                nc.scalar.copy(h8_sb[:, :, g * P:(g + 1) * P],
                               hT_sb[:, :, g * P:(g + 1) * P])
        return h8_sb

    def qk_chunk(qk_sb, hT_sb, fs):
        cq, ck = C_QKV * HD ** -0.5, C_QKV
        for f in fs:
            ps = pp_qkv.tile([P, TB], f32, tag="big")
            for t in range(2):
                nc.tensor.matmul(
                    ps[:], wqkv_sb[:, 2 * t:2 * t + 2, f * P:(f + 1) * P],
                    hT_sb[:, 2 * t:2 * t + 2, :],
                    start=(t == 0), stop=(t == 1), perf_mode=DR)
            qi, ci = divmod(f, 4)
            nc.any.tensor_scalar(qk_sb[:, qi, ci], ps[:],
                                 cq if qi == 0 else ck,
                                 bqkc_sb[:, f:f + 1], ALU.mult, ALU.add)

    def v_chunk(v_sb, hT_sb, gs):
        for g in gs:
            ps = pp_qkv.tile([P, C], f32, tag="big")
            for t in range(2):
                nc.tensor.matmul(
                    ps[:], hT_sb[:, 2 * t:2 * t + 2, g * P:(g + 1) * P],
                    wqkv_sb[:, 2 * t:2 * t + 2, 2 * C:3 * C],
                    start=(t == 0), stop=(t == 1), perf_mode=DR)
            # v stays 2^13-scaled (descale folded into ONES_VAL)
            nc.any.tensor_tensor(v_sb[:, g], ps[:], bvbc_sb[:], ALU.add)

    def emit_qkv(hT_sb):
        qk_sb = qkpool.tile([P, 2, 4, TB], bf16, tag="qk")
        qk_chunk(qk_sb, hT_sb, range(8))
        v_sb = vpool.tile([P, NG, C], bf16, tag="v")
        v_chunk(v_sb, hT_sb, range(NG))
        return qk_sb, v_sb

    def attn_cc(cc, qk_sb, v_sb, oT_sb, pe_filler):
        # QK^T for 4 heads, one PSUM bank per head-tile i (concurrent
        # row-packed matmuls must drain into distinct banks); only the
        # first 256 fp32 of each bank are used.
        pss = pp_s.tile([P, 4, TB], f32, tag="pss")
        psv = pss[:, :, 0:NG * 64].rearrange("p i (g q) -> p i g q", g=NG)
        for w in range(8):
            pairi, wcl = divmod(w, 2)
            for i in range(4):
                out_sl = (psv[0:64, i, pairi] if wcl == 0
                          else psv[64:128, i, pairi])
                nc.tensor.matmul(
                    out_sl,
                    qk_sb[32 * i:32 * (i + 1), 1, cc, w * 64:(w + 1) * 64],
                    qk_sb[32 * i:32 * (i + 1), 0, cc, w * 64:(w + 1) * 64],
                    start=True, stop=True,
                    tile_position=(32 * i, 0 if wcl == 0 else 64))
        # bias+exp priority-boosted: on the ACT/DVE queues they must jump
        # ahead of the filler relu evacs and LN-chain casts emitted around
        # them, since exp(cc) gates the next cc's QK via the psv slot
        with tc.high_priority(offset=250):
            nc.vector.tensor_tensor(
                psv[:], psv[:],
                biasT_sb[:, 4 * cc:4 * cc + 4, None, :].to_broadcast(
                    (P, 4, NG, 64)),
                ALU.add)
            nc.scalar.activation(scores_sb[0:64, 4 * cc:4 * cc + 4, :, 0, :],
                                 psv[0:64], AF.Exp)
            nc.scalar.activation(scores_sb[64:128, 4 * cc:4 * cc + 4, :, 1, :],
                                 psv[64:128], AF.Exp)
        # PE filler work (mlp of band-2) lands here in the PE stream, so
        # the engine computes through the bias+exp chain instead of
        # stalling on the pd/AV matmuls that need the fresh scores
        pe_filler()
        # softmax denominators: col-packed ones-matmuls + fast reciprocal
        pd = pp_qkv.tile([P, NG, P], f32, tag="big")
        for pair in range(NG):
            for j in range(4):
                nc.tensor.matmul(
                    pd[32 * j:32 * (j + 1), pair], ones_sb[:],
                    scores_sb[:, 4 * cc + j, pair],
                    start=True, stop=True, tile_position=(0, 32 * j))
        rd = rdpool.tile([P, NG, P], f32, tag="rd")
        with tc.high_priority(offset=250):
            nc.vector.reciprocal_approx_fast(rd[:], pd[:])
        # AV col-packed 4 heads into one PSUM bank; normalize (and the
        # 2^13->2^8 rescale, via ONES_VAL) on the PSUM->SBUF evacuation
        # multiply, writing fp8 directly
        po = pp_qkv.tile([P, NG, P], f32, tag="big")
        for pair in range(NG):
            for j in range(4):
                nc.tensor.matmul(
                    po[32 * j:32 * (j + 1), pair],
                    v_sb[:, pair, (4 * cc + j) * HD:(4 * cc + j + 1) * HD],
                    scores_sb[:, 4 * cc + j, pair],
                    start=True, stop=True, tile_position=(0, 32 * j))
        with tc.high_priority(offset=250):
            nc.vector.tensor_tensor(oT_sb[:, cc], po[:], rd[:], ALU.mult)

    def mlp1_chunk(m1_sb, h2T_sb, fcs):
        for fc in fcs:
            ps = pp_mlp.tile([P, TB], f32, tag="mlp")
            for t in range(2):
                nc.tensor.matmul(
                    ps[:], w1_sb[:, 2 * t:2 * t + 2, fc, :],
                    h2T_sb[:, 2 * t:2 * t + 2, :],
                    start=(t == 0), stop=(t == 1), perf_mode=DR)
            # psum is 2^13-scaled; bias column is host-scaled by S_M so
            # the evac writes S_M*relu(true) as fp8 in one op
            nc.scalar.activation(m1_sb[:, fc], ps[:], AF.Relu,
                                 bias=bm1c_sb[:, fc:fc + 1], scale=C_M1)

    def mlp2_g(prev, g):
        m1_sb, y_sb, pband = prev
        ps = pp_mlp.tile([P, C], f32, tag="mlp")
        for t in range(8):
            nc.tensor.matmul(
                ps[:], m1_sb[:, 2 * t:2 * t + 2, g * P:(g + 1) * P],
                w2_sb[:, 2 * t:2 * t + 2, :],
                start=(t == 0), stop=(t == 7), perf_mode=DR)
        t_sb = ptpool.tile([P, C], f32, tag="mt")
        o_sb = outpool.tile([P, C], f32, tag="out")
        nc.any.tensor_scalar_mul(t_sb[:], ps[:], C_MLP2)
        nc.any.tensor_tensor(o_sb[:], t_sb[:], y_sb[:, g], ALU.add)
        nc.any.tensor_tensor(o_sb[:], o_sb[:], bm2bc_sb[:], ALU.add)
        # out stores on the sync HWDGE ring (not SWDGE: xbar conflict;
        # hwdge copies were observed NOT to block transposes)
        for wcl in range(2):
            nc.sync.dma_start(out_r[pband, g, wcl],
                              o_sb[wcl * 64:(wcl + 1) * 64])

    # ---- prologue: band 0 LN1 runs BEFORE the bulk weight loads so its
    # x loads and transposes aren't starved of HBM bandwidth / xbar turns;
    # only the q,k weights (which gate the first GEMM) go first. The LN1
    # chain runs TWO bands ahead of its qkv consumer (the apply/transpose/
    # cast chain is ~15us of serial latency incl. xbar-vs-DMA waits; one
    # band of lookahead wasn't enough to hide it once fp8 shrank the PE
    # work per band), so the prologue also builds hT8(1) ----
    nc.sync.dma_start(wqkv_sb[:, :, :2 * C], wqkv_r[:, :, :2 * C])
    xs = {}
    x_sb, mv1 = load_stats(0, prologue=True)
    xs[0] = x_sb
    # band-0 chain FIRST: its transposes serialize against every
    # in-flight DMA, so x(1) and the weight bulk are emitted after it
    hT8 = {0: ln_apply_transpose(x_sb, ln_rstd(mv1, "1"), "hT")}
    load_weights()
    qk_sb, v_sb = emit_qkv(hT8[0])
    xs[1], mv1b = load_stats(1)
    hT8[1] = ln_apply_transpose(xs[1], ln_rstd(mv1b, "1"), "hT")
    pending = {}  # band -> (h2T8, y, band): MLP not yet run

    for band in range(NB):
        # band+2's x loads + LN stats go first: the rings and DVE
        # have slack at the start of the attention block
        if band + 2 < NB:
            xs[band + 2], nxt_mv = load_stats(band + 2)

        # ---- attention(band), interleaving mlp1(band-2) + mlp2(band-2)
        # as PE fillers. The MLP trails its band by TWO bands so the
        # LN2 chain (proj evac -> stats -> rstd -> apply -> xbar
        # transpose -> cast, ~15-20us of serial latency incl. the
        # transpose-vs-in-flight-DMA waits) has a full band of slack
        # before the mlp1 fillers consume h2T8. The four chunks are
        # mlp1 halves then mlp2 halves (mlp2 needs all of m1, which
        # fillers 0-1 complete) ----
        x_sb = xs[band]
        oT_sb = opool.tile([P, 4, TB], f8, tag="oT")
        if band - 2 in pending:
            h2T_prev, y_prev, pband = pending.pop(band - 2)
            m1_sb = m1pool.tile([P, 16, TB], f8, tag="m1")
            pm = (m1_sb, y_prev, pband)
            fillers = [lambda: mlp1_chunk(m1_sb, h2T_prev, range(0, 8)),
                       lambda: mlp1_chunk(m1_sb, h2T_prev, range(8, 16)),
                       lambda: (mlp2_g(pm, 0), mlp2_g(pm, 1)),
                       lambda: (mlp2_g(pm, 2), mlp2_g(pm, 3))]
        else:
            fillers = [lambda: None] * 4
        attn_cc(0, qk_sb, v_sb, oT_sb, fillers[0])
        attn_cc(1, qk_sb, v_sb, oT_sb, fillers[1])
        if band + 2 < NB:
            nxt_ab = ln_rstd(nxt_mv, "1")
        attn_cc(2, qk_sb, v_sb, oT_sb, fillers[2])
        # LN1(band+2) applies + transposes emitted one attention-quarter
        # early: the chain was landing ~8us after qkv(band+2) wanted it
        # (qkv's first LDWEIGHTS stalls on the last cast chunk), so shift
        # its queue positions forward while attn_cc(3) covers on the PE
        if band + 2 < NB:
            hT8[band + 2] = ln_apply_transpose(xs[band + 2], nxt_ab, "hT")
        attn_cc(3, qk_sb, v_sb, oT_sb, fillers[3])

        # ---- proj(band) + residual + LN2 stats ----
        st2 = spool.tile([P, NG, 6], f32, tag="st2", name="st2")
        mv2 = spool.tile([P, NG, 2], f32, tag="mv2", name="mv2")
        for g in range(NG):
            ps = pp_mlp.tile([P, C], f32, tag="mlp")
            for t in range(2):
                nc.tensor.matmul(
                    ps[:], oT_sb[:, 2 * t:2 * t + 2, g * P:(g + 1) * P],
                    wproj_sb[:, 2 * t:2 * t + 2, :],
                    start=(t == 0), stop=(t == 1), perf_mode=DR)
            t_sb = ptpool.tile([P, C], f32, tag="pt")
            nc.any.tensor_scalar_mul(t_sb[:], ps[:], C_PROJ)
            nc.any.tensor_tensor(x_sb[:, g], t_sb[:], x_sb[:, g], ALU.add)
            nc.any.tensor_tensor(x_sb[:, g], x_sb[:, g], bpbc_sb[:], ALU.add)
            nc.vector.bn_stats(out=st2[:, g], in_=x_sb[:, g])
            nc.vector.bn_aggr(out=mv2[:, g], in_=st2[:, g])
        y_sb = x_sb

        # ---- qkv(band+1) keeps the PE stream unbroken while the LN2
        # chain below runs on DVE/ACT/sync ----
        if band + 1 < NB:
            qk_sb, v_sb = emit_qkv(hT8[band + 1])
        h2T_sb = ln_apply_transpose(y_sb, ln_rstd(mv2, "2"), "h2T")
        if band == 0:
            # MLP-era weights behind T2(0) in the sync FIFO: ready well
            # before attention(2)'s mlp1(0) fillers, without delaying the
            # T1/T2 transposes behind weight traffic
            load_weights_mlp()

        pending[band] = (h2T_sb, y_sb, band)

    # ---- epilogue: the last two bands' MLPs (their attention cover is
    # gone); mlp(NB-2) starts immediately and hides the LN2(NB-1) chain ----
    for pband in (NB - 2, NB - 1):
        h2T_p, y_p, _ = pending.pop(pband)
        m1_sb = m1pool.tile([P, 16, TB], f8, tag="m1")
        mlp1_chunk(m1_sb, h2T_p, range(16))
        for g in range(NG):
            mlp2_g((m1_sb, y_p, pband), g)


@functools.lru_cache(maxsize=2)
def _build():
    from contextlib import ExitStack
    import concourse.mybir as mybir
    import concourse.tile as tile
    from concourse import bacc

    dt = mybir.dt
    nc = bacc.Bacc("TRN2", target_bir_lowering=False, debug=False,
                   num_devices=N_CORES)
    aps = {}
    specs = [
        ("x", [H, W, C], dt.float32),
        ("wqkv", [C, 3 * C], dt.float8e4),
        ("wproj", [C, C], dt.float8e4),
        ("w1", [C, 4 * C], dt.float8e4),
        ("w2", [4 * C, C], dt.float8e4),
        ("bqkc", [P, 8], dt.float32),
        ("bm1c", [P, 16], dt.float32),
        ("bvbc", [P, C], dt.bfloat16),
        ("bpbc", [P, C], dt.bfloat16),
        ("bm2bc", [P, C], dt.bfloat16),
        ("biasT", [P, NH, 64], dt.bfloat16),
    ]
    for name, shape, dtype in specs:
        aps[name] = nc.dram_tensor(name, shape, dtype,
                                   kind="ExternalInput").ap()
    aps["out"] = nc.dram_tensor("out", [H, W, C], dt.float32,
                                kind="ExternalOutput").ap()
    with tile.TileContext(nc) as tc:
        with ExitStack() as ctx:
            _emit(nc, tc, ctx, aps)
    nc.compile()
    return nc


def _prepare_in_maps(x, g1, b1, wqkv, bqkv, wproj, bproj, rel_bias, g2, b2,
                     w1, bm1, w2, bm2):
    x = np.asarray(x, np.float32)
    f = lambda a: np.ascontiguousarray(np.asarray(a, np.float32))
    g1, b1, wqkv, bqkv = f(g1), f(b1), f(wqkv), f(bqkv)
    wproj, bproj, rel_bias = f(wproj), f(bproj), f(rel_bias)
    g2, b2, w1, bm1, w2, bm2 = f(g2), f(b2), f(w1), f(bm1), f(w2), f(bm2)

    # fold LN1 affine into wqkv/bqkv. The attention scale HD^-0.5 is NOT
    # folded into the q weights (that would shift their fp8 binades for
    # nothing) -- it rides the q evac descale constant instead; the bias
    # columns DO carry it since they're added post-descale.
    wqkv_f = g1[:, None] * wqkv
    bqkv_f = b1 @ wqkv + bqkv
    sc = HD ** -0.5
    bqkv_f[:C] *= sc
    # fold LN2 affine into w1/bm1
    w1_f = g2[:, None] * w1
    bm1_f = b2 @ w1 + bm1

    bqkc = np.ascontiguousarray(bqkv_f[:2 * C].reshape(8, P).T)   # [128, 8]
    # bm1 is added inside the relu evac, post-descale but pre-S_M-rescale
    bm1c = np.ascontiguousarray((bm1_f * S_M).reshape(16, P).T)   # [128, 16]
    import ml_dtypes
    bfarr = lambda a: np.ascontiguousarray(a).astype(ml_dtypes.bfloat16)
    fp8arr = lambda a: np.clip(np.ascontiguousarray(a) * S_W, -240.0,
                               240.0).astype(ml_dtypes.float8_e4m3)
    # v bias is 2^13-scaled: v lives in 2^13 units until the softmax-
    # normalize multiply (ONES_VAL folds the descale)
    bvbc = bfarr(np.broadcast_to(bqkv_f[2 * C:] * (S_A * S_W), (P, C)))
    bpbc = bfarr(np.broadcast_to(bproj, (P, C)))
    bm2bc = bfarr(np.broadcast_to(bm2, (P, C)))

    idx = _rel_pos_index()                              # [64(n), 64(m)]
    bias_nm = rel_bias[idx, :]                          # [n, m, NH]
    biasT_h = bias_nm.transpose(2, 1, 0)                # [NH, m, n]
    biasT = np.concatenate([biasT_h, biasT_h], axis=1)  # [NH, 128, 64]
    biasT = bfarr(biasT.transpose(1, 0, 2))             # [128, NH, 64]

    wqkv_b, wproj_b, w1_b, w2_b = (fp8arr(wqkv_f), fp8arr(wproj),
                                   fp8arr(w1_f), fp8arr(w2))
    shared = dict(wqkv=wqkv_b, wproj=wproj_b, w1=w1_b, w2=w2_b,
                  bqkc=bqkc, bm1c=bm1c, bvbc=bvbc, bpbc=bpbc, bm2bc=bm2bc,
                  biasT=biasT)
    return [dict(x=np.ascontiguousarray(x[c]), **shared)
            for c in range(N_CORES)]


def kernel(**inputs):
    from concourse.bass_utils import run_bass_kernel_spmd

    in_maps = _prepare_in_maps(**inputs)
    nc = _build()
    res = run_bass_kernel_spmd(nc, in_maps, core_ids=list(range(N_CORES)))
    return np.stack([res.results[c]["out"] for c in range(N_CORES)], axis=0)
